# revision 9
# baseline (speedup 1.0000x reference)
"""Trainium2 Bass kernel for nn_AE_RNN: 2-layer GRU AE-RNN, scalar SSE loss.

Data-parallel over batch (256 -> 32 per core, 8 cores). Host sums partials.

v3: merged-layer lockstep recurrence. Iteration i computes layer0 step i
("A", cols 0:32) and layer1 step i-1 ("B", cols 32:64) in shared [128,64]
elementwise ops. Gates carry both a negated-z (zc = 1-z) and positive-z
(z') sigmoid so the blend h = zc*n + z'*h_prev needs no 1-x op. Gate
input biases preload into PSUM via identity matmuls; per-step chain is
  3 MMs -> sigmoid(r) -> tn -> tnb -> tanh -> aa -> h'
with the n-PSUM copied to SBUF off-chain so tn/tnb run in DVE 2x mode.
Bulk gi passes use DVE tensor_scalar (bias AP); tail ReLUs use DVE
scalar_tensor_tensor; bulk/tail emission is interleaved into the step
loop so engine FIFOs never stall at chunk boundaries.
"""

import numpy as np
import ml_dtypes

import concourse.bass as bass
import concourse.bacc as bacc
import concourse.mybir as mybir
import concourse.tile as tile
from concourse.bass_utils import run_bass_kernel_spmd

BF16 = mybir.dt.bfloat16
F32 = mybir.dt.float32
AF = mybir.ActivationFunctionType
OP = mybir.AluOpType

B, T = 256, 2048
U = Y = 32
H = 128
Z = 64
NCORE = 8
BL = B // NCORE  # 32 batch rows per core

C = 128  # steps per chunk
NCH = T // C


def _bf(x):
    return np.asarray(x, dtype=ml_dtypes.bfloat16)


def build_nc(nch=NCH):
    nc = bacc.Bacc("TRN2", target_bir_lowering=False, debug=False)
    Tl = nch * C
    NT = (C * BL) // 512  # 512-wide tiles per chunk = 8

    u_d = nc.dram_tensor("u", [U, Tl, BL], BF16, kind="ExternalInput")
    y_d = nc.dram_tensor("y", [Y, Tl, BL], BF16, kind="ExternalInput")
    h00_d = nc.dram_tensor("h00", [H, BL], F32, kind="ExternalInput")
    h01_d = nc.dram_tensor("h01", [H, BL], F32, kind="ExternalInput")
    wp_d = nc.dram_tensor("wpack", [128, 23 * 128], BF16, kind="ExternalInput")
    bp_d = nc.dram_tensor("bpack", [128, 10], F32, kind="ExternalInput")
    out_d = nc.dram_tensor("out", [1, 1], F32, kind="ExternalOutput")

    with tile.TileContext(nc) as tc:
        with (
            tc.tile_pool(name="wt", bufs=1) as wtp,
            tc.tile_pool(name="ut", bufs=2) as utp,
            tc.tile_pool(name="yt", bufs=3) as ytp,
            tc.tile_pool(name="big", bufs=2) as bigp,
            tc.tile_pool(name="r1", bufs=3) as r1p,
            tc.tile_pool(name="hs", bufs=3) as hsp,
            tc.tile_pool(name="sm", bufs=4) as smp,
            tc.tile_pool(name="tl", bufs=2) as tlp,
            tc.tile_pool(name="prz", bufs=2, space="PSUM") as przp,
            tc.tile_pool(name="pn", bufs=2, space="PSUM") as pnp,
            tc.tile_pool(name="psb", bufs=2, space="PSUM") as psb,
        ):
            wp = wtp.tile([128, 23 * 128], BF16)
            bp = wtp.tile([128, 10], F32)
            nc.sync.dma_start(wp[:], wp_d[:])
            nc.sync.dma_start(bp[:], bp_d[:])

            def W(i, n=128):
                return wp[:, i * 128:i * 128 + n]

            def Bc(j, p=128):
                return bp[:p, j:j + 1]

            I128 = W(18)

            ones32 = wtp.tile([32, 1], F32)
            nc.vector.memset(ones32[:], 1.0)
            zeros512 = wtp.tile([128, 512], BF16)
            nc.vector.memset(zeros512[:], 0.0)

            h0f = wtp.tile([H, BL], F32)
            h1f = wtp.tile([H, BL], F32)
            nc.sync.dma_start(h0f[:], h00_d[:])
            nc.sync.dma_start(h1f[:], h01_d[:])

            loss_acc = wtp.tile([32, NT * nch], F32)
            nc.vector.memset(loss_acc[:], 0.0)

            # ---------- bulk building blocks (emitted interleaved) ----------
            def relu1_tile(relu1, Ut, j):
                ps = psb.tile([128, 512], F32, tag="pb")
                nc.tensor.matmul(ps[:], W(0)[0:32, :],
                                 Ut[:, j * 512:(j + 1) * 512])
                nc.scalar.activation(relu1[:, j * 512:(j + 1) * 512], ps[:],
                                     AF.Relu, bias=Bc(0))

            def gi_tile(gb, relu1, wi, bi, j):
                ps = psb.tile([128, 512], F32, tag="pb")
                nc.tensor.matmul(ps[:], W(wi),
                                 relu1[:, j * 512:(j + 1) * 512])
                nc.vector.tensor_scalar(gb[:, j * 512:(j + 1) * 512], ps[:],
                                        Bc(bi), None, op0=OP.add)

            def tail_tile(prev, k, j):
                relu1, HS_t, Yt = prev
                hsv = HS_t[:].rearrange("p (t x) -> p t x", x=64)
                sl = slice(j * 512, (j + 1) * 512)
                h1rhs = hsv[:, j * 16 + 1:j * 16 + 17, 32:64]
                ps = psb.tile([128, 512], F32, tag="pb")
                nc.tensor.matmul(ps[:], W(13), relu1[:, sl],
                                 start=True, stop=False)
                nc.tensor.matmul(ps[:], W(14), h1rhs, start=False, stop=True)
                dphi = tlp.tile([128, 512], BF16, tag="dphi")
                nc.vector.scalar_tensor_tensor(dphi[:], ps[:], Bc(4),
                                               zeros512[:], op0=OP.add,
                                               op1=OP.max)
                ps2 = psb.tile([128, 512], F32, tag="pb")
                nc.tensor.matmul(ps2[:], W(15), dphi[:])
                px = tlp.tile([128, 512], BF16, tag="px")
                nc.vector.scalar_tensor_tensor(px[:], ps2[:], Bc(5),
                                               zeros512[:], op0=OP.add,
                                               op1=OP.max)
                ps3 = psb.tile([128, 512], F32, tag="pb")
                nc.tensor.matmul(ps3[:], W(16), px[:])
                me = tlp.tile([128, 512], BF16, tag="me")
                nc.vector.scalar_tensor_tensor(me[:], ps3[:], Bc(6),
                                               zeros512[:], op0=OP.add,
                                               op1=OP.max)
                ps4t = psb.tile([128, 512], F32, tag="pb")
                ps4 = ps4t[0:32, :]
                nc.tensor.matmul(ps4, W(17, 32), me[:])
                d = tlp.tile([32, 512], F32, tag="dd")
                nc.vector.scalar_tensor_tensor(d[:], ps4, Bc(7, 32),
                                               Yt[:, sl], op0=OP.add,
                                               op1=OP.subtract)
                d2 = tlp.tile([32, 512], F32, tag="d2")
                nc.scalar.activation(
                    d2[:], d[:], AF.Square,
                    accum_out=loss_acc[:, k * NT + j:k * NT + j + 1])

            # ---------- prologue: chunk 0 inputs + bulk ----------
            Ut = utp.tile([U, C * BL], BF16, tag="ut")
            Yt = ytp.tile([Y, C * BL], BF16, tag="yt")
            nc.sync.dma_start(Ut[:].rearrange("p (t b) -> p t b", b=BL),
                              u_d[:, 0:C, :])
            nc.sync.dma_start(Yt[:].rearrange("p (t b) -> p t b", b=BL),
                              y_d[:, 0:C, :])
            relu1 = r1p.tile([H, C * BL], BF16, tag="relu1")
            G0R = bigp.tile([H, C * BL], BF16, tag="g0r")
            G0Z = bigp.tile([H, C * BL], BF16, tag="g0z")
            G0P = bigp.tile([H, C * BL], BF16, tag="g0p")
            G0N = bigp.tile([H, C * BL], BF16, tag="g0n")
            for j in range(NT):
                relu1_tile(relu1, Ut, j)
            for j in range(NT):
                gi_tile(G0R, relu1, 1, 1, j)
                gi_tile(G0Z, relu1, 2, 2, j)
                gi_tile(G0P, relu1, 19, 8, j)
                gi_tile(G0N, relu1, 3, 3, j)

            HS_prev_tile = None
            prev = None  # (relu1, HS, Yt) of previous chunk for tail

            for k in range(nch):
                HS = hsp.tile([H, (C + 1) * 64], BF16, tag="hs")
                if k == 0:
                    nc.vector.tensor_copy(HS[:, 0:32], h0f[:])
                    nc.vector.tensor_copy(HS[:, 96:128], h1f[:])

                last = k + 1 >= nch
                if not last:
                    Ut_n = utp.tile([U, C * BL], BF16, tag="ut")
                    Yt_n = ytp.tile([Y, C * BL], BF16, tag="yt")
                    nc.sync.dma_start(
                        Ut_n[:].rearrange("p (t b) -> p t b", b=BL),
                        u_d[:, (k + 1) * C:(k + 2) * C, :])
                    nc.sync.dma_start(
                        Yt_n[:].rearrange("p (t b) -> p t b", b=BL),
                        y_d[:, (k + 1) * C:(k + 2) * C, :])
                    relu1_n = r1p.tile([H, C * BL], BF16, tag="relu1")
                    G0R_n = bigp.tile([H, C * BL], BF16, tag="g0r")
                    G0Z_n = bigp.tile([H, C * BL], BF16, tag="g0z")
                    G0P_n = bigp.tile([H, C * BL], BF16, tag="g0p")
                    G0N_n = bigp.tile([H, C * BL], BF16, tag="g0n")

                for s in range(C):
                    i = k * C + s  # global iteration
                    first = (i == 0)
                    if s == 0:
                        hprev = (HS[:, 0:64] if k == 0 else
                                 HS_prev_tile[:, C * 64:(C + 1) * 64])
                    else:
                        hprev = HS[:, s * 64:(s + 1) * 64]
                    h0p = hprev[:, 0:32]
                    h1p = hprev[:, 32:64]
                    s32 = slice(s * 32, (s + 1) * 32)

                    # --- PSUM rz: [rA rB zcA zcB zpA zpB] ---
                    rz = przp.tile([128, 192], F32, tag="rz")
                    nps = pnp.tile([128, 128], F32, tag="n")
                    # preloads (no h dependency; run early)
                    nc.tensor.matmul(rz[:, 0:32], I128, G0R[:, s32],
                                     start=True, stop=False)
                    nc.tensor.matmul(rz[:, 64:96], I128, G0Z[:, s32],
                                     start=True, stop=False)
                    nc.tensor.matmul(rz[:, 128:160], I128, G0P[:, s32],
                                     start=True, stop=False)
                    nc.tensor.matmul(nps[:, 64:96], I128, G0N[:, s32])
                    # h-dependent: r first (chain), then n, then z
                    nc.tensor.matmul(rz[:, 0:32], W(4), h0p,
                                     start=False, stop=True)
                    if not first:
                        nc.tensor.matmul(rz[:, 32:64], W(7), h0p,
                                         start=True, stop=False)
                        nc.tensor.matmul(rz[:, 32:64], W(10), h1p,
                                         start=False, stop=True)
                    nc.tensor.matmul(nps[:, 0:32], W(6), h0p)
                    if not first:
                        nc.tensor.matmul(nps[:, 32:64], W(12), h1p)
                        nc.tensor.matmul(nps[:, 96:128], W(9), h0p)
                    nc.tensor.matmul(rz[:, 64:96], W(5), h0p,
                                     start=False, stop=True)
                    nc.tensor.matmul(rz[:, 128:160], W(20), h0p,
                                     start=False, stop=True)
                    if not first:
                        nc.tensor.matmul(rz[:, 96:128], W(8), h0p,
                                         start=True, stop=False)
                        nc.tensor.matmul(rz[:, 96:128], W(11), h1p,
                                         start=False, stop=True)
                        nc.tensor.matmul(rz[:, 160:192], W(21), h0p,
                                         start=True, stop=False)
                        nc.tensor.matmul(rz[:, 160:192], W(22), h1p,
                                         start=False, stop=True)

                    wd = 32 if first else 64
                    # --- sigmoids ---
                    rs = smp.tile([128, 64], BF16, tag="rs")
                    nc.scalar.activation(rs[:, 0:wd], rz[:, 0:wd], AF.Sigmoid)
                    zs = smp.tile([128, 128], BF16, tag="zs")
                    if first:
                        nc.scalar.activation(zs[:, 0:32], rz[:, 64:96],
                                             AF.Sigmoid)
                        nc.scalar.activation(zs[:, 64:96], rz[:, 128:160],
                                             AF.Sigmoid)
                    else:
                        nc.scalar.activation(zs[:], rz[:, 64:192], AF.Sigmoid)
                    # --- n path (nc copied to SBUF off-chain) ---
                    nsb = smp.tile([128, 128], BF16, tag="nsb")
                    if first:
                        nc.vector.tensor_copy(nsb[:, 0:32], nps[:, 0:32])
                        nc.vector.tensor_copy(nsb[:, 64:96], nps[:, 64:96])
                    else:
                        nc.vector.tensor_copy(nsb[:], nps[:])
                    tn = smp.tile([128, 64], BF16, tag="tn")
                    nc.vector.tensor_tensor(tn[:, 0:wd], rs[:, 0:wd],
                                            nsb[:, 0:wd], op=OP.mult)
                    tnb = smp.tile([128, 64], BF16, tag="tnb")
                    nc.vector.tensor_tensor(tnb[:, 0:wd], tn[:, 0:wd],
                                            nsb[:, 64:64 + wd], op=OP.add)
                    ns = smp.tile([128, 64], BF16, tag="ns")
                    nc.scalar.activation(ns[:, 0:wd], tnb[:, 0:wd], AF.Tanh)
                    # --- blend: h = zc*n + z'*h_prev ---
                    bb = smp.tile([128, 64], BF16, tag="bb")
                    nc.gpsimd.tensor_tensor(bb[:, 0:wd], zs[:, 64:64 + wd],
                                            hprev[:, 0:wd], op=OP.mult)
                    aa = smp.tile([128, 64], BF16, tag="aa")
                    nc.vector.tensor_tensor(aa[:, 0:wd], zs[:, 0:wd],
                                            ns[:, 0:wd], op=OP.mult)
                    dst = HS[:, (s + 1) * 64:(s + 1) * 64 + wd]
                    nc.vector.tensor_tensor(dst, aa[:, 0:wd], bb[:, 0:wd],
                                            op=OP.add)

                    # --- interleaved bulk (chunk k+1) and tail (chunk k-1) ---
                    j, ph = divmod(s - 4, 16)
                    if 0 <= j < NT:
                        if ph == 0 and not last:
                            relu1_tile(relu1_n, Ut_n, j)
                        elif ph == 2 and not last:
                            gi_tile(G0R_n, relu1_n, 1, 1, j)
                        elif ph == 4 and not last:
                            gi_tile(G0Z_n, relu1_n, 2, 2, j)
                        elif ph == 6 and not last:
                            gi_tile(G0P_n, relu1_n, 19, 8, j)
                        elif ph == 8 and not last:
                            gi_tile(G0N_n, relu1_n, 3, 3, j)
                        elif ph == 10 and prev is not None:
                            tail_tile(prev, k - 1, j)

                prev = (relu1, HS, Yt)
                HS_prev_tile = HS
                if not last:
                    relu1 = relu1_n
                    G0R, G0Z, G0P, G0N = G0R_n, G0Z_n, G0P_n, G0N_n
                    Ut, Yt = Ut_n, Yt_n

            for j in range(NT):
                tail_tile(prev, nch - 1, j)

            # ---- final loss reduction ----
            lsum = smp.tile([32, 1], F32, tag="lsum")
            nc.vector.tensor_reduce(lsum[:], loss_acc[:],
                                    axis=mybir.AxisListType.X, op=OP.add)
            pslt = psb.tile([128, 512], F32, tag="pb")
            psl = pslt[0:1, 0:1]
            nc.tensor.matmul(psl, ones32[:], lsum[:])
            lout = smp.tile([1, 1], F32, tag="lout")
            nc.vector.tensor_copy(lout[:], psl)
            nc.sync.dma_start(out_d[:], lout[:])

    nc.compile()
    return nc


def prep_inputs(u, y, h0, pu_w1, pu_b1, pu_w2, pu_b2, dy_w1, dy_b1, dy_w2,
                dy_b2, xm_w, xm_b, xlv_w, xlv_b, px_w1, px_b1, px_w2, px_b2,
                me_w1, me_b1, me_w2, me_b2, gru_wih, gru_whh, t_steps=T):
    """Host-side: compose weights, build wpack/bpack, per-core input shards."""
    f = np.float32
    asf = lambda x: np.array(x, f)
    (u, y, h0, pu_w1, pu_b1, pu_w2, pu_b2, dy_w1, dy_b1, dy_w2, dy_b2, xm_w,
     xm_b, xlv_w, xlv_b, px_w1, px_b1, px_w2, px_b2, me_w1, me_b1, me_w2,
     me_b2, gru_wih, gru_whh) = map(asf, (
         u, y, h0, pu_w1, pu_b1, pu_w2, pu_b2, dy_w1, dy_b1, dy_w2, dy_b2,
         xm_w, xm_b, xlv_w, xlv_b, px_w1, px_b1, px_w2, px_b2, me_w1, me_b1,
         me_w2, me_b2, gru_wih, gru_whh))
    wih0p, wih1p = gru_wih[0], gru_wih[1]
    whh0p, whh1p = gru_whh[0], gru_whh[1]
    # negated z-gate copies (sigmoid gives zc = 1-z)
    def zneg(w):
        w = w.copy()
        w[H:2 * H] *= -1.0
        return w
    wih0, wih1 = zneg(wih0p), zneg(wih1p)
    whh0, whh1 = zneg(whh0p), zneg(whh1p)

    Wg0 = wih0 @ pu_w2           # [3H, H], z rows negated
    bg0 = wih0 @ pu_b2
    Wg0p_z = wih0p[H:2 * H] @ pu_w2   # positive z
    bg0p_z = wih0p[H:2 * H] @ pu_b2
    Wda = dy_w1[:, :H] @ pu_w2
    bda = dy_w1[:, :H] @ pu_b2 + dy_b1
    dyw1b = dy_w1[:, H:]
    Wxc = np.vstack([xm_w, xlv_w])
    WxcP = Wxc @ dy_w2
    bxcP = Wxc @ dy_b2 + np.concatenate([xm_b, xlv_b])
    Wpx = px_w1 @ WxcP
    bpx = px_w1 @ bxcP + px_b1
    WmeP = me_w1 @ px_w2
    bmeP = me_w1 @ px_b2 + me_b1

    wpack = np.zeros((128, 23 * 128), f)

    def put(i, w):  # w: [out, in] -> lhsT [in, out]
        wt = np.ascontiguousarray(w.T)
        wpack[:wt.shape[0], i * 128:i * 128 + wt.shape[1]] = wt

    put(0, pu_w1)
    for g in range(3):
        put(1 + g, Wg0[g * H:(g + 1) * H])
        put(4 + g, whh0[g * H:(g + 1) * H])
        put(7 + g, wih1[g * H:(g + 1) * H])
        put(10 + g, whh1[g * H:(g + 1) * H])
    put(13, Wda)
    put(14, dyw1b)
    put(15, Wpx)
    put(16, WmeP)
    put(17, me_w2)
    put(18, np.eye(128, dtype=f))
    put(19, Wg0p_z)
    put(20, whh0p[H:2 * H])
    put(21, wih1p[H:2 * H])
    put(22, whh1p[H:2 * H])

    bpack = np.zeros((128, 10), f)
    for j, b in enumerate([pu_b1, bg0[:H], bg0[H:2 * H], bg0[2 * H:],
                           bda, bpx, bmeP, me_b2, bg0p_z]):
        bpack[:len(b), j] = b

    in_maps = []
    for c in range(NCORE):
        rows = slice(c * BL, (c + 1) * BL)
        in_maps.append({
            "u": _bf(u[rows, :, :t_steps].transpose(1, 2, 0)),  # [U, T, BL]
            "y": _bf(y[rows, :, :t_steps].transpose(1, 2, 0)),
            "h00": np.ascontiguousarray(h0[0, rows].T),
            "h01": np.ascontiguousarray(h0[1, rows].T),
            "wpack": _bf(wpack),
            "bpack": bpack,
        })
    return in_maps


_NC_CACHE = {}


def run_on_cores(in_maps, nch=NCH, trace=False, tmpdir=None):
    key = nch
    if key not in _NC_CACHE:
        _NC_CACHE[key] = build_nc(nch)
    nc = _NC_CACHE[key]
    res = run_bass_kernel_spmd(nc, in_maps, core_ids=list(range(NCORE)),
                               trace=trace, tmpdir=tmpdir)
    total = np.float32(0.0)
    for r in res.results:
        total += np.float32(r["out"][0, 0])
    return np.float32(total), res


def kernel(**inputs):
    in_maps = prep_inputs(**inputs)
    total, _ = run_on_cores(in_maps)
    return total


# revision 10
# speedup vs baseline: 1.1991x; 1.1991x over previous
"""Trainium2 Bass kernel for nn_AE_RNN: 2-layer GRU AE-RNN, scalar SSE loss.

Data-parallel over batch (256 -> 32 per core, 8 cores). Host sums partials.

v3: merged-layer lockstep recurrence. Iteration i computes layer0 step i
("A", cols 0:32) and layer1 step i-1 ("B", cols 32:64) in shared [128,64]
elementwise ops. Gates carry both a negated-z (zc = 1-z) and positive-z
(z') sigmoid so the blend h = zc*n + z'*h_prev needs no 1-x op. Gate
input biases preload into PSUM via identity matmuls; per-step chain is
  3 MMs -> sigmoid(r) -> tn -> tnb -> tanh -> aa -> h'
with the n-PSUM copied to SBUF off-chain so tn/tnb run in DVE 2x mode.
Bulk gi passes use DVE tensor_scalar (bias AP); tail ReLUs use DVE
scalar_tensor_tensor; bulk/tail emission is interleaved into the step
loop so engine FIFOs never stall at chunk boundaries.
"""

import numpy as np
import ml_dtypes

import concourse.bass as bass
import concourse.bacc as bacc
import concourse.mybir as mybir
import concourse.tile as tile
from concourse.bass_utils import run_bass_kernel_spmd

BF16 = mybir.dt.bfloat16
F32 = mybir.dt.float32
AF = mybir.ActivationFunctionType
OP = mybir.AluOpType

B, T = 256, 2048
U = Y = 32
H = 128
Z = 64
NCORE = 8
BL = B // NCORE  # 32 batch rows per core

C = 128  # steps per chunk
NCH = T // C


def _bf(x):
    return np.asarray(x, dtype=ml_dtypes.bfloat16)


def build_nc(nch=NCH):
    nc = bacc.Bacc("TRN2", target_bir_lowering=False, debug=False)
    Tl = nch * C
    NT = (C * BL) // 512  # 512-wide tiles per chunk = 8

    u_d = nc.dram_tensor("u", [U, Tl, BL], BF16, kind="ExternalInput")
    y_d = nc.dram_tensor("y", [Y, Tl, BL], BF16, kind="ExternalInput")
    h00_d = nc.dram_tensor("h00", [H, BL], F32, kind="ExternalInput")
    h01_d = nc.dram_tensor("h01", [H, BL], F32, kind="ExternalInput")
    wp_d = nc.dram_tensor("wpack", [128, 23 * 128], BF16, kind="ExternalInput")
    bp_d = nc.dram_tensor("bpack", [128, 10], F32, kind="ExternalInput")
    out_d = nc.dram_tensor("out", [1, 1], F32, kind="ExternalOutput")

    with tile.TileContext(nc) as tc:
        with (
            tc.tile_pool(name="wt", bufs=1) as wtp,
            tc.tile_pool(name="ut", bufs=2) as utp,
            tc.tile_pool(name="yt", bufs=3) as ytp,
            tc.tile_pool(name="big", bufs=2) as bigp,
            tc.tile_pool(name="r1", bufs=3) as r1p,
            tc.tile_pool(name="hs", bufs=3) as hsp,
            tc.tile_pool(name="sm", bufs=4) as smp,
            tc.tile_pool(name="tl", bufs=2) as tlp,
            tc.tile_pool(name="prz", bufs=2, space="PSUM") as przp,
            tc.tile_pool(name="pn", bufs=2, space="PSUM") as pnp,
            tc.tile_pool(name="psb", bufs=2, space="PSUM") as psb,
        ):
            wp = wtp.tile([128, 23 * 128], BF16)
            bp = wtp.tile([128, 10], F32)
            nc.sync.dma_start(wp[:], wp_d[:])
            nc.sync.dma_start(bp[:], bp_d[:])

            def W(i, n=128):
                return wp[:, i * 128:i * 128 + n]

            def Bc(j, p=128):
                return bp[:p, j:j + 1]

            I128 = W(18)

            ones32 = wtp.tile([32, 1], F32)
            nc.vector.memset(ones32[:], 1.0)
            zeros512 = wtp.tile([128, 512], BF16)
            nc.vector.memset(zeros512[:], 0.0)

            h0f = wtp.tile([H, BL], F32)
            h1f = wtp.tile([H, BL], F32)
            nc.sync.dma_start(h0f[:], h00_d[:])
            nc.sync.dma_start(h1f[:], h01_d[:])

            loss_acc = wtp.tile([32, NT * nch], F32)
            nc.vector.memset(loss_acc[:], 0.0)

            # ---------- bulk building blocks (emitted interleaved) ----------
            def relu1_tile(relu1, Ut, j):
                ps = psb.tile([128, 512], F32, tag="pb")
                nc.tensor.matmul(ps[:], W(0)[0:32, :],
                                 Ut[:, j * 512:(j + 1) * 512])
                nc.scalar.activation(relu1[:, j * 512:(j + 1) * 512], ps[:],
                                     AF.Relu, bias=Bc(0))

            def gi_tile(gb, relu1, wi, bi, j):
                ps = psb.tile([128, 512], F32, tag="pb")
                nc.tensor.matmul(ps[:], W(wi),
                                 relu1[:, j * 512:(j + 1) * 512])
                nc.vector.tensor_scalar(gb[:, j * 512:(j + 1) * 512], ps[:],
                                        Bc(bi), None, op0=OP.add)

            def tail_a(prev, j):
                relu1, HS_t, Yt = prev
                hsv = HS_t[:].rearrange("p (t x) -> p t x", x=64)
                sl = slice(j * 512, (j + 1) * 512)
                h1rhs = hsv[:, j * 16 + 1:j * 16 + 17, 32:64]
                ps = psb.tile([128, 512], F32, tag="pb")
                nc.tensor.matmul(ps[:], W(13), relu1[:, sl],
                                 start=True, stop=False)
                nc.tensor.matmul(ps[:], W(14), h1rhs, start=False, stop=True)
                dphi = tlp.tile([128, 512], BF16, tag="dphi")
                nc.vector.scalar_tensor_tensor(dphi[:], ps[:], Bc(4),
                                               zeros512[:], op0=OP.add,
                                               op1=OP.max)
                return dphi

            def tail_b(dphi):
                ps2 = psb.tile([128, 512], F32, tag="pb")
                nc.tensor.matmul(ps2[:], W(15), dphi[:])
                px = tlp.tile([128, 512], BF16, tag="px")
                nc.vector.scalar_tensor_tensor(px[:], ps2[:], Bc(5),
                                               zeros512[:], op0=OP.add,
                                               op1=OP.max)
                return px

            def tail_c(px):
                ps3 = psb.tile([128, 512], F32, tag="pb")
                nc.tensor.matmul(ps3[:], W(16), px[:])
                me = tlp.tile([128, 512], BF16, tag="me")
                nc.vector.scalar_tensor_tensor(me[:], ps3[:], Bc(6),
                                               zeros512[:], op0=OP.add,
                                               op1=OP.max)
                return me

            def tail_d(me, prev, k, j):
                _, _, Yt = prev
                sl = slice(j * 512, (j + 1) * 512)
                ps4t = psb.tile([128, 512], F32, tag="pb")
                ps4 = ps4t[0:32, :]
                nc.tensor.matmul(ps4, W(17, 32), me[:])
                d = tlp.tile([32, 512], F32, tag="dd")
                nc.vector.scalar_tensor_tensor(d[:], ps4, Bc(7, 32),
                                               Yt[:, sl], op0=OP.add,
                                               op1=OP.subtract)
                d2 = tlp.tile([32, 512], F32, tag="d2")
                nc.scalar.activation(
                    d2[:], d[:], AF.Square,
                    accum_out=loss_acc[:, k * NT + j:k * NT + j + 1])

            def tail_tile(prev, k, j):
                tail_d(tail_c(tail_b(tail_a(prev, j))), prev, k, j)

            # ---------- prologue: chunk 0 inputs + bulk ----------
            Ut = utp.tile([U, C * BL], BF16, tag="ut")
            Yt = ytp.tile([Y, C * BL], BF16, tag="yt")
            nc.sync.dma_start(Ut[:].rearrange("p (t b) -> p t b", b=BL),
                              u_d[:, 0:C, :])
            nc.sync.dma_start(Yt[:].rearrange("p (t b) -> p t b", b=BL),
                              y_d[:, 0:C, :])
            relu1 = r1p.tile([H, C * BL], BF16, tag="relu1")
            G0R = bigp.tile([H, C * BL], BF16, tag="g0r")
            G0Z = bigp.tile([H, C * BL], BF16, tag="g0z")
            G0P = bigp.tile([H, C * BL], BF16, tag="g0p")
            G0N = bigp.tile([H, C * BL], BF16, tag="g0n")
            for j in range(NT):
                relu1_tile(relu1, Ut, j)
            for j in range(NT):
                gi_tile(G0R, relu1, 1, 1, j)
                gi_tile(G0Z, relu1, 2, 2, j)
                gi_tile(G0P, relu1, 19, 8, j)
                gi_tile(G0N, relu1, 3, 3, j)

            HS_prev_tile = None
            prev = None  # (relu1, HS, Yt) of previous chunk for tail

            for k in range(nch):
                HS = hsp.tile([H, (C + 1) * 64], BF16, tag="hs")
                if k == 0:
                    nc.vector.tensor_copy(HS[:, 0:32], h0f[:])
                    nc.vector.tensor_copy(HS[:, 96:128], h1f[:])

                last = k + 1 >= nch
                if not last:
                    Ut_n = utp.tile([U, C * BL], BF16, tag="ut")
                    Yt_n = ytp.tile([Y, C * BL], BF16, tag="yt")
                    nc.sync.dma_start(
                        Ut_n[:].rearrange("p (t b) -> p t b", b=BL),
                        u_d[:, (k + 1) * C:(k + 2) * C, :])
                    nc.sync.dma_start(
                        Yt_n[:].rearrange("p (t b) -> p t b", b=BL),
                        y_d[:, (k + 1) * C:(k + 2) * C, :])
                    relu1_n = r1p.tile([H, C * BL], BF16, tag="relu1")
                    G0R_n = bigp.tile([H, C * BL], BF16, tag="g0r")
                    G0Z_n = bigp.tile([H, C * BL], BF16, tag="g0z")
                    G0P_n = bigp.tile([H, C * BL], BF16, tag="g0p")
                    G0N_n = bigp.tile([H, C * BL], BF16, tag="g0n")

                for s in range(C):
                    i = k * C + s  # global iteration
                    first = (i == 0)
                    if s == 0:
                        hprev = (HS[:, 0:64] if k == 0 else
                                 HS_prev_tile[:, C * 64:(C + 1) * 64])
                    else:
                        hprev = HS[:, s * 64:(s + 1) * 64]
                    h0p = hprev[:, 0:32]
                    h1p = hprev[:, 32:64]
                    s32 = slice(s * 32, (s + 1) * 32)

                    # --- PSUM rz: [rA rB zcA zcB zpA zpB] ---
                    rz = przp.tile([128, 192], F32, tag="rz")
                    nps = pnp.tile([128, 128], F32, tag="n")
                    # preloads (no h dependency; run early)
                    nc.tensor.matmul(rz[:, 0:32], I128, G0R[:, s32],
                                     start=True, stop=False)
                    nc.tensor.matmul(rz[:, 64:96], I128, G0Z[:, s32],
                                     start=True, stop=False)
                    nc.tensor.matmul(rz[:, 128:160], I128, G0P[:, s32],
                                     start=True, stop=False)
                    nc.tensor.matmul(nps[:, 64:96], I128, G0N[:, s32])
                    # h-dependent: r first (chain), then n, then z
                    nc.tensor.matmul(rz[:, 0:32], W(4), h0p,
                                     start=False, stop=True)
                    if not first:
                        nc.tensor.matmul(rz[:, 32:64], W(7), h0p,
                                         start=True, stop=False)
                        nc.tensor.matmul(rz[:, 32:64], W(10), h1p,
                                         start=False, stop=True)
                    nc.tensor.matmul(nps[:, 0:32], W(6), h0p)
                    if not first:
                        nc.tensor.matmul(nps[:, 32:64], W(12), h1p)
                        nc.tensor.matmul(nps[:, 96:128], W(9), h0p)
                    nc.tensor.matmul(rz[:, 64:96], W(5), h0p,
                                     start=False, stop=True)
                    nc.tensor.matmul(rz[:, 128:160], W(20), h0p,
                                     start=False, stop=True)
                    if not first:
                        nc.tensor.matmul(rz[:, 96:128], W(8), h0p,
                                         start=True, stop=False)
                        nc.tensor.matmul(rz[:, 96:128], W(11), h1p,
                                         start=False, stop=True)
                        nc.tensor.matmul(rz[:, 160:192], W(21), h0p,
                                         start=True, stop=False)
                        nc.tensor.matmul(rz[:, 160:192], W(22), h1p,
                                         start=False, stop=True)

                    wd = 32 if first else 64
                    # --- sigmoids ---
                    rs = smp.tile([128, 64], BF16, tag="rs")
                    nc.scalar.activation(rs[:, 0:wd], rz[:, 0:wd], AF.Sigmoid)
                    zs = smp.tile([128, 128], BF16, tag="zs")
                    if first:
                        nc.scalar.activation(zs[:, 0:32], rz[:, 64:96],
                                             AF.Sigmoid)
                        nc.scalar.activation(zs[:, 64:96], rz[:, 128:160],
                                             AF.Sigmoid)
                    else:
                        nc.scalar.activation(zs[:], rz[:, 64:192], AF.Sigmoid)
                    # --- n path (nc copied to SBUF off-chain) ---
                    nsb = smp.tile([128, 128], BF16, tag="nsb")
                    if first:
                        nc.vector.tensor_copy(nsb[:, 0:32], nps[:, 0:32])
                        nc.vector.tensor_copy(nsb[:, 64:96], nps[:, 64:96])
                    else:
                        nc.vector.tensor_copy(nsb[:], nps[:])
                    tn = smp.tile([128, 64], BF16, tag="tn")
                    nc.vector.tensor_tensor(tn[:, 0:wd], rs[:, 0:wd],
                                            nsb[:, 0:wd], op=OP.mult)
                    tnb = smp.tile([128, 64], BF16, tag="tnb")
                    nc.vector.tensor_tensor(tnb[:, 0:wd], tn[:, 0:wd],
                                            nsb[:, 64:64 + wd], op=OP.add)
                    ns = smp.tile([128, 64], BF16, tag="ns")
                    nc.scalar.activation(ns[:, 0:wd], tnb[:, 0:wd], AF.Tanh)
                    # --- blend: h = zc*n + z'*h_prev ---
                    bb = smp.tile([128, 64], BF16, tag="bb")
                    nc.gpsimd.tensor_tensor(bb[:, 0:wd], zs[:, 64:64 + wd],
                                            hprev[:, 0:wd], op=OP.mult)
                    aa = smp.tile([128, 64], BF16, tag="aa")
                    nc.vector.tensor_tensor(aa[:, 0:wd], zs[:, 0:wd],
                                            ns[:, 0:wd], op=OP.mult)
                    dst = HS[:, (s + 1) * 64:(s + 1) * 64 + wd]
                    nc.vector.tensor_tensor(dst, aa[:, 0:wd], bb[:, 0:wd],
                                            op=OP.add)

                    # --- interleaved bulk (chunk k+1) and tail (chunk k-1) ---
                    j, ph = divmod(s - 4, 16)
                    if 0 <= j < NT:
                        if ph == 0 and not last:
                            relu1_tile(relu1_n, Ut_n, j)
                        elif ph == 2 and not last:
                            gi_tile(G0R_n, relu1_n, 1, 1, j)
                        elif ph == 4 and not last:
                            gi_tile(G0Z_n, relu1_n, 2, 2, j)
                        elif ph == 6 and not last:
                            gi_tile(G0P_n, relu1_n, 19, 8, j)
                        elif ph == 8 and not last:
                            gi_tile(G0N_n, relu1_n, 3, 3, j)
                        elif ph == 10 and prev is not None:
                            tail_tile(prev, k - 1, j)

                prev = (relu1, HS, Yt)
                HS_prev_tile = HS
                if not last:
                    relu1 = relu1_n
                    G0R, G0Z, G0P, G0N = G0R_n, G0Z_n, G0P_n, G0N_n
                    Ut, Yt = Ut_n, Yt_n

            for j in range(NT):
                tail_tile(prev, nch - 1, j)

            # ---- final loss reduction ----
            lsum = smp.tile([32, 1], F32, tag="lsum")
            nc.vector.tensor_reduce(lsum[:], loss_acc[:],
                                    axis=mybir.AxisListType.X, op=OP.add)
            pslt = psb.tile([128, 512], F32, tag="pb")
            psl = pslt[0:1, 0:1]
            nc.tensor.matmul(psl, ones32[:], lsum[:])
            lout = smp.tile([1, 1], F32, tag="lout")
            nc.vector.tensor_copy(lout[:], psl)
            nc.sync.dma_start(out_d[:], lout[:])

    nc.compile()
    return nc


def prep_inputs(u, y, h0, pu_w1, pu_b1, pu_w2, pu_b2, dy_w1, dy_b1, dy_w2,
                dy_b2, xm_w, xm_b, xlv_w, xlv_b, px_w1, px_b1, px_w2, px_b2,
                me_w1, me_b1, me_w2, me_b2, gru_wih, gru_whh, t_steps=T):
    """Host-side: compose weights, build wpack/bpack, per-core input shards."""
    f = np.float32
    asf = lambda x: np.array(x, f)
    (u, y, h0, pu_w1, pu_b1, pu_w2, pu_b2, dy_w1, dy_b1, dy_w2, dy_b2, xm_w,
     xm_b, xlv_w, xlv_b, px_w1, px_b1, px_w2, px_b2, me_w1, me_b1, me_w2,
     me_b2, gru_wih, gru_whh) = map(asf, (
         u, y, h0, pu_w1, pu_b1, pu_w2, pu_b2, dy_w1, dy_b1, dy_w2, dy_b2,
         xm_w, xm_b, xlv_w, xlv_b, px_w1, px_b1, px_w2, px_b2, me_w1, me_b1,
         me_w2, me_b2, gru_wih, gru_whh))
    wih0p, wih1p = gru_wih[0], gru_wih[1]
    whh0p, whh1p = gru_whh[0], gru_whh[1]
    # negated z-gate copies (sigmoid gives zc = 1-z)
    def zneg(w):
        w = w.copy()
        w[H:2 * H] *= -1.0
        return w
    wih0, wih1 = zneg(wih0p), zneg(wih1p)
    whh0, whh1 = zneg(whh0p), zneg(whh1p)

    Wg0 = wih0 @ pu_w2           # [3H, H], z rows negated
    bg0 = wih0 @ pu_b2
    Wg0p_z = wih0p[H:2 * H] @ pu_w2   # positive z
    bg0p_z = wih0p[H:2 * H] @ pu_b2
    Wda = dy_w1[:, :H] @ pu_w2
    bda = dy_w1[:, :H] @ pu_b2 + dy_b1
    dyw1b = dy_w1[:, H:]
    Wxc = np.vstack([xm_w, xlv_w])
    WxcP = Wxc @ dy_w2
    bxcP = Wxc @ dy_b2 + np.concatenate([xm_b, xlv_b])
    Wpx = px_w1 @ WxcP
    bpx = px_w1 @ bxcP + px_b1
    WmeP = me_w1 @ px_w2
    bmeP = me_w1 @ px_b2 + me_b1

    wpack = np.zeros((128, 23 * 128), f)

    def put(i, w):  # w: [out, in] -> lhsT [in, out]
        wt = np.ascontiguousarray(w.T)
        wpack[:wt.shape[0], i * 128:i * 128 + wt.shape[1]] = wt

    put(0, pu_w1)
    for g in range(3):
        put(1 + g, Wg0[g * H:(g + 1) * H])
        put(4 + g, whh0[g * H:(g + 1) * H])
        put(7 + g, wih1[g * H:(g + 1) * H])
        put(10 + g, whh1[g * H:(g + 1) * H])
    put(13, Wda)
    put(14, dyw1b)
    put(15, Wpx)
    put(16, WmeP)
    put(17, me_w2)
    put(18, np.eye(128, dtype=f))
    put(19, Wg0p_z)
    put(20, whh0p[H:2 * H])
    put(21, wih1p[H:2 * H])
    put(22, whh1p[H:2 * H])

    bpack = np.zeros((128, 10), f)
    for j, b in enumerate([pu_b1, bg0[:H], bg0[H:2 * H], bg0[2 * H:],
                           bda, bpx, bmeP, me_b2, bg0p_z]):
        bpack[:len(b), j] = b

    in_maps = []
    for c in range(NCORE):
        rows = slice(c * BL, (c + 1) * BL)
        in_maps.append({
            "u": _bf(u[rows, :, :t_steps].transpose(1, 2, 0)),  # [U, T, BL]
            "y": _bf(y[rows, :, :t_steps].transpose(1, 2, 0)),
            "h00": np.ascontiguousarray(h0[0, rows].T),
            "h01": np.ascontiguousarray(h0[1, rows].T),
            "wpack": _bf(wpack),
            "bpack": bpack,
        })
    return in_maps


_NC_CACHE = {}


def run_on_cores(in_maps, nch=NCH, trace=False, tmpdir=None):
    key = nch
    if key not in _NC_CACHE:
        _NC_CACHE[key] = build_nc(nch)
    nc = _NC_CACHE[key]
    res = run_bass_kernel_spmd(nc, in_maps, core_ids=list(range(NCORE)),
                               trace=trace, tmpdir=tmpdir)
    total = np.float32(0.0)
    for r in res.results:
        total += np.float32(r["out"][0, 0])
    return np.float32(total), res


def kernel(**inputs):
    in_maps = prep_inputs(**inputs)
    total, _ = run_on_cores(in_maps)
    return total


# revision 16
# speedup vs baseline: 1.2809x; 1.0682x over previous
"""Trainium2 Bass kernel for nn_AE_RNN: 2-layer GRU AE-RNN, scalar SSE loss.

Data-parallel over batch (256 -> 32 per core, 8 cores). Host sums partials.

v3: merged-layer lockstep recurrence. Iteration i computes layer0 step i
("A", cols 0:32) and layer1 step i-1 ("B", cols 32:64) in shared [128,64]
elementwise ops. Gates carry both a negated-z (zc = 1-z) and positive-z
(z') sigmoid so the blend h = zc*n + z'*h_prev needs no 1-x op. Gate
input biases preload into PSUM via identity matmuls; per-step chain is
  3 MMs -> sigmoid(r) -> tn -> tnb -> tanh -> aa -> h'
with the n-PSUM copied to SBUF off-chain so tn/tnb run in DVE 2x mode.
Bulk gi passes use DVE tensor_scalar (bias AP); tail ReLUs use DVE
scalar_tensor_tensor; bulk/tail emission is interleaved into the step
loop so engine FIFOs never stall at chunk boundaries.
"""

import numpy as np
import ml_dtypes

import concourse.bass as bass
import concourse.bacc as bacc
import concourse.mybir as mybir
import concourse.tile as tile
from concourse.bass_utils import run_bass_kernel_spmd

BF16 = mybir.dt.bfloat16
F32 = mybir.dt.float32
AF = mybir.ActivationFunctionType
OP = mybir.AluOpType

B, T = 256, 2048
U = Y = 32
H = 128
Z = 64
NCORE = 8
BL = B // NCORE  # 32 batch rows per core

C = 128  # steps per chunk
NCH = T // C


def _bf(x):
    return np.asarray(x, dtype=ml_dtypes.bfloat16)


def build_nc(nch=NCH):
    nc = bacc.Bacc("TRN2", target_bir_lowering=False, debug=False)
    Tl = nch * C
    NT = (C * BL) // 512  # 512-wide tiles per chunk = 8

    u_d = nc.dram_tensor("u", [U, Tl, BL], BF16, kind="ExternalInput")
    y_d = nc.dram_tensor("y", [Y, Tl, BL], BF16, kind="ExternalInput")
    h00_d = nc.dram_tensor("h00", [H, BL], F32, kind="ExternalInput")
    h01_d = nc.dram_tensor("h01", [H, BL], F32, kind="ExternalInput")
    wp_d = nc.dram_tensor("wpack", [128, 23 * 128], BF16, kind="ExternalInput")
    bp_d = nc.dram_tensor("bpack", [128, 10], F32, kind="ExternalInput")
    out_d = nc.dram_tensor("out", [1, 1], F32, kind="ExternalOutput")

    with tile.TileContext(nc) as tc:
        with (
            tc.tile_pool(name="wt", bufs=1) as wtp,
            tc.tile_pool(name="ut", bufs=2) as utp,
            tc.tile_pool(name="yt", bufs=3) as ytp,
            tc.tile_pool(name="big", bufs=2) as bigp,
            tc.tile_pool(name="r1", bufs=3) as r1p,
            tc.tile_pool(name="hs", bufs=3) as hsp,
            tc.tile_pool(name="sm", bufs=4) as smp,
            tc.tile_pool(name="tl", bufs=2) as tlp,
            tc.tile_pool(name="prz", bufs=2, space="PSUM") as przp,
            tc.tile_pool(name="pn", bufs=2, space="PSUM") as pnp,
            tc.tile_pool(name="psb", bufs=2, space="PSUM") as psb,
        ):
            wp = wtp.tile([128, 23 * 128], BF16)
            bp = wtp.tile([128, 10], F32)
            nc.sync.dma_start(wp[:], wp_d[:])
            nc.sync.dma_start(bp[:], bp_d[:])

            def W(i, n=128):
                return wp[:, i * 128:i * 128 + n]

            def Bc(j, p=128):
                return bp[:p, j:j + 1]

            I128 = W(18)

            ones32 = wtp.tile([32, 1], F32)
            nc.vector.memset(ones32[:], 1.0)
            zeros512 = wtp.tile([128, 512], BF16)
            nc.vector.memset(zeros512[:], 0.0)

            h0f = wtp.tile([H, BL], F32)
            h1f = wtp.tile([H, BL], F32)
            nc.sync.dma_start(h0f[:], h00_d[:])
            nc.sync.dma_start(h1f[:], h01_d[:])

            loss_acc = wtp.tile([32, NT * nch], F32)
            nc.vector.memset(loss_acc[:], 0.0)

            # ---------- bulk building blocks (emitted interleaved) ----------
            def relu1_tile(relu1, Ut, j):
                ps = psb.tile([128, 512], F32, tag="pb")
                nc.tensor.matmul(ps[:], W(0)[0:32, :],
                                 Ut[:, j * 512:(j + 1) * 512])
                nc.scalar.activation(relu1[:, j * 512:(j + 1) * 512], ps[:],
                                     AF.Relu, bias=Bc(0))

            def gi_tile(gb, relu1, wi, bi, j):
                ps = psb.tile([128, 512], F32, tag="pb")
                nc.tensor.matmul(ps[:], W(wi),
                                 relu1[:, j * 512:(j + 1) * 512])
                nc.vector.tensor_scalar(gb[:, j * 512:(j + 1) * 512], ps[:],
                                        Bc(bi), None, op0=OP.add)

            def tail_a(prev, j):
                relu1, HS_t, Yt = prev
                hsv = HS_t[:].rearrange("p (t x) -> p t x", x=64)
                sl = slice(j * 512, (j + 1) * 512)
                h1rhs = hsv[:, j * 16 + 1:j * 16 + 17, 32:64]
                ps = psb.tile([128, 512], F32, tag="pb")
                nc.tensor.matmul(ps[:], W(13), relu1[:, sl],
                                 start=True, stop=False)
                nc.tensor.matmul(ps[:], W(14), h1rhs, start=False, stop=True)
                dphi = tlp.tile([128, 512], BF16, tag="dphi")
                nc.vector.scalar_tensor_tensor(dphi[:], ps[:], Bc(4),
                                               zeros512[:], op0=OP.add,
                                               op1=OP.max)
                return dphi

            def tail_b(dphi):
                ps2 = psb.tile([128, 512], F32, tag="pb")
                nc.tensor.matmul(ps2[:], W(15), dphi[:])
                px = tlp.tile([128, 512], BF16, tag="px")
                nc.vector.scalar_tensor_tensor(px[:], ps2[:], Bc(5),
                                               zeros512[:], op0=OP.add,
                                               op1=OP.max)
                return px

            def tail_c(px):
                ps3 = psb.tile([128, 512], F32, tag="pb")
                nc.tensor.matmul(ps3[:], W(16), px[:])
                me = tlp.tile([128, 512], BF16, tag="me")
                nc.vector.scalar_tensor_tensor(me[:], ps3[:], Bc(6),
                                               zeros512[:], op0=OP.add,
                                               op1=OP.max)
                return me

            def tail_d(me, prev, k, j):
                _, _, Yt = prev
                sl = slice(j * 512, (j + 1) * 512)
                ps4t = psb.tile([128, 512], F32, tag="pb")
                ps4 = ps4t[0:32, :]
                nc.tensor.matmul(ps4, W(17, 32), me[:])
                d = tlp.tile([32, 512], F32, tag="dd")
                nc.vector.scalar_tensor_tensor(d[:], ps4, Bc(7, 32),
                                               Yt[:, sl], op0=OP.add,
                                               op1=OP.subtract)
                d2 = tlp.tile([32, 512], F32, tag="d2")
                nc.scalar.activation(
                    d2[:], d[:], AF.Square,
                    accum_out=loss_acc[:, k * NT + j:k * NT + j + 1])

            def tail_tile(prev, k, j):
                tail_d(tail_c(tail_b(tail_a(prev, j))), prev, k, j)

            # ---------- prologue: chunk 0 inputs + bulk ----------
            Ut = utp.tile([U, C * BL], BF16, tag="ut")
            Yt = ytp.tile([Y, C * BL], BF16, tag="yt")
            nc.sync.dma_start(Ut[:].rearrange("p (t b) -> p t b", b=BL),
                              u_d[:, 0:C, :])
            nc.sync.dma_start(Yt[:].rearrange("p (t b) -> p t b", b=BL),
                              y_d[:, 0:C, :])
            relu1 = r1p.tile([H, C * BL], BF16, tag="relu1")
            G0R = bigp.tile([H, C * BL], BF16, tag="g0r")
            G0Z = bigp.tile([H, C * BL], BF16, tag="g0z")
            G0P = bigp.tile([H, C * BL], BF16, tag="g0p")
            G0N = bigp.tile([H, C * BL], BF16, tag="g0n")
            for j in range(NT):
                relu1_tile(relu1, Ut, j)
            for j in range(NT):
                gi_tile(G0R, relu1, 1, 1, j)
                gi_tile(G0Z, relu1, 2, 2, j)
                gi_tile(G0P, relu1, 19, 8, j)
                gi_tile(G0N, relu1, 3, 3, j)

            HS_prev_tile = None
            prev = None  # (relu1, HS, Yt) of previous chunk for tail
            aap = bbp = None

            for k in range(nch):
                HS = hsp.tile([H, (C + 1) * 64], BF16, tag="hs")
                if k == 0:
                    nc.vector.tensor_copy(HS[:, 0:32], h0f[:])
                    nc.vector.tensor_copy(HS[:, 96:128], h1f[:])

                last = k + 1 >= nch
                if not last:
                    Ut_n = utp.tile([U, C * BL], BF16, tag="ut")
                    Yt_n = ytp.tile([Y, C * BL], BF16, tag="yt")
                    nc.sync.dma_start(
                        Ut_n[:].rearrange("p (t b) -> p t b", b=BL),
                        u_d[:, (k + 1) * C:(k + 2) * C, :])
                    nc.sync.dma_start(
                        Yt_n[:].rearrange("p (t b) -> p t b", b=BL),
                        y_d[:, (k + 1) * C:(k + 2) * C, :])
                    relu1_n = r1p.tile([H, C * BL], BF16, tag="relu1")
                    G0R_n = bigp.tile([H, C * BL], BF16, tag="g0r")
                    G0Z_n = bigp.tile([H, C * BL], BF16, tag="g0z")
                    G0P_n = bigp.tile([H, C * BL], BF16, tag="g0p")
                    G0N_n = bigp.tile([H, C * BL], BF16, tag="g0n")

                for s in range(C):
                    i = k * C + s  # global iteration
                    first = (i == 0)
                    if s == 0:
                        hprev = (HS[:, 0:64] if k == 0 else
                                 HS_prev_tile[:, C * 64:(C + 1) * 64])
                    else:
                        hprev = HS[:, s * 64:(s + 1) * 64]
                    h0p = hprev[:, 0:32]
                    h1p = hprev[:, 32:64]
                    s32 = slice(s * 32, (s + 1) * 32)

                    # --- PSUM rz: [rA rB zcA zcB zpA zpB] ---
                    rz = przp.tile([128, 192], F32, tag="rz")
                    nps = pnp.tile([128, 128], F32, tag="n")
                    # preloads (no h dependency; run early)
                    nc.tensor.matmul(rz[:, 0:32], I128, G0R[:, s32],
                                     start=True, stop=False)
                    nc.tensor.matmul(rz[:, 64:96], I128, G0Z[:, s32],
                                     start=True, stop=False)
                    nc.tensor.matmul(rz[:, 128:160], I128, G0P[:, s32],
                                     start=True, stop=False)
                    nc.tensor.matmul(nps[:, 64:96], I128, G0N[:, s32])
                    # h-dependent: r first (chain), then n, then z.
                    # For i>=2 the r-gate consumes aa/bb of the previous step
                    # directly (W@h' = W@aa + W@bb) so the h'-add is off-chain.
                    if i >= 2:
                        nc.tensor.matmul(rz[:, 0:32], W(4), aap[:, 0:32],
                                         start=False, stop=False)
                        nc.tensor.matmul(rz[:, 0:32], W(4), bbp[:, 0:32],
                                         start=False, stop=True)
                        nc.tensor.matmul(rz[:, 32:64], W(7), aap[:, 0:32],
                                         start=True, stop=False)
                        nc.tensor.matmul(rz[:, 32:64], W(7), bbp[:, 0:32],
                                         start=False, stop=False)
                        nc.tensor.matmul(rz[:, 32:64], W(10), aap[:, 32:64],
                                         start=False, stop=False)
                        nc.tensor.matmul(rz[:, 32:64], W(10), bbp[:, 32:64],
                                         start=False, stop=True)
                    else:
                        nc.tensor.matmul(rz[:, 0:32], W(4), h0p,
                                         start=False, stop=True)
                        if not first:
                            nc.tensor.matmul(rz[:, 32:64], W(7), h0p,
                                             start=True, stop=False)
                            nc.tensor.matmul(rz[:, 32:64], W(10), h1p,
                                             start=False, stop=True)
                    nc.tensor.matmul(nps[:, 0:32], W(6), h0p)
                    if not first:
                        nc.tensor.matmul(nps[:, 32:64], W(12), h1p)
                        nc.tensor.matmul(nps[:, 96:128], W(9), h0p)
                    nc.tensor.matmul(rz[:, 64:96], W(5), h0p,
                                     start=False, stop=True)
                    nc.tensor.matmul(rz[:, 128:160], W(20), h0p,
                                     start=False, stop=True)
                    if not first:
                        nc.tensor.matmul(rz[:, 96:128], W(8), h0p,
                                         start=True, stop=False)
                        nc.tensor.matmul(rz[:, 96:128], W(11), h1p,
                                         start=False, stop=True)
                        nc.tensor.matmul(rz[:, 160:192], W(21), h0p,
                                         start=True, stop=False)
                        nc.tensor.matmul(rz[:, 160:192], W(22), h1p,
                                         start=False, stop=True)

                    wd = 32 if first else 64
                    # --- sigmoids ---
                    rs = smp.tile([128, 64], BF16, tag="rs")
                    nc.scalar.activation(rs[:, 0:wd], rz[:, 0:wd], AF.Sigmoid)
                    zs = smp.tile([128, 128], BF16, tag="zs")
                    if first:
                        nc.scalar.activation(zs[:, 0:32], rz[:, 64:96],
                                             AF.Sigmoid)
                        nc.scalar.activation(zs[:, 64:96], rz[:, 128:160],
                                             AF.Sigmoid)
                    else:
                        nc.scalar.activation(zs[:], rz[:, 64:192], AF.Sigmoid)
                    # --- n path ---
                    tn = smp.tile([128, 64], BF16, tag="tn")
                    nc.vector.tensor_tensor(tn[:, 0:wd], rs[:, 0:wd],
                                            nps[:, 0:wd], op=OP.mult)
                    tnb = smp.tile([128, 64], BF16, tag="tnb")
                    nc.vector.tensor_tensor(tnb[:, 0:wd], tn[:, 0:wd],
                                            nps[:, 64:64 + wd], op=OP.add)
                    ns = smp.tile([128, 64], BF16, tag="ns")
                    nc.scalar.activation(ns[:, 0:wd], tnb[:, 0:wd], AF.Tanh)
                    # --- blend: h = zc*n + z'*h_prev ---
                    bb = smp.tile([128, 64], BF16, tag="bb")
                    nc.gpsimd.tensor_tensor(bb[:, 0:wd], zs[:, 64:64 + wd],
                                            hprev[:, 0:wd], op=OP.mult)
                    aa = smp.tile([128, 64], BF16, tag="aa")
                    nc.vector.tensor_tensor(aa[:, 0:wd], zs[:, 0:wd],
                                            ns[:, 0:wd], op=OP.mult)
                    dst = HS[:, (s + 1) * 64:(s + 1) * 64 + wd]
                    nc.vector.tensor_tensor(dst, aa[:, 0:wd], bb[:, 0:wd],
                                            op=OP.add)
                    aap, bbp = aa, bb

                    # --- interleaved bulk (chunk k+1) and tail (chunk k-1) ---
                    j, ph = divmod(s - 4, 16)
                    if 0 <= j < NT:
                        if ph == 0 and not last:
                            relu1_tile(relu1_n, Ut_n, j)
                        elif ph == 2 and not last:
                            gi_tile(G0R_n, relu1_n, 1, 1, j)
                        elif ph == 4 and not last:
                            gi_tile(G0Z_n, relu1_n, 2, 2, j)
                        elif ph == 6 and not last:
                            gi_tile(G0P_n, relu1_n, 19, 8, j)
                        elif ph == 8 and not last:
                            gi_tile(G0N_n, relu1_n, 3, 3, j)
                        elif ph == 9 and prev is not None:
                            t_dphi = tail_a(prev, j)
                        elif ph == 11 and prev is not None:
                            t_px = tail_b(t_dphi)
                        elif ph == 13 and prev is not None:
                            t_me = tail_c(t_px)  # never fires for j=NT-1
                        elif ph == 15 and prev is not None:
                            tail_d(t_me, prev, k - 1, j)

                # last tile's tail_c/tail_d fall past s=127; emit here
                if prev is not None:
                    t_me = tail_c(t_px)
                    tail_d(t_me, prev, k - 1, NT - 1)

                prev = (relu1, HS, Yt)
                HS_prev_tile = HS
                if not last:
                    relu1 = relu1_n
                    G0R, G0Z, G0P, G0N = G0R_n, G0Z_n, G0P_n, G0N_n
                    Ut, Yt = Ut_n, Yt_n

            for j in range(NT):
                tail_tile(prev, nch - 1, j)

            # ---- final loss reduction ----
            lsum = smp.tile([32, 1], F32, tag="lsum")
            nc.vector.tensor_reduce(lsum[:], loss_acc[:],
                                    axis=mybir.AxisListType.X, op=OP.add)
            pslt = psb.tile([128, 512], F32, tag="pb")
            psl = pslt[0:1, 0:1]
            nc.tensor.matmul(psl, ones32[:], lsum[:])
            lout = smp.tile([1, 1], F32, tag="lout")
            nc.vector.tensor_copy(lout[:], psl)
            nc.sync.dma_start(out_d[:], lout[:])

    nc.compile()
    return nc


def prep_inputs(u, y, h0, pu_w1, pu_b1, pu_w2, pu_b2, dy_w1, dy_b1, dy_w2,
                dy_b2, xm_w, xm_b, xlv_w, xlv_b, px_w1, px_b1, px_w2, px_b2,
                me_w1, me_b1, me_w2, me_b2, gru_wih, gru_whh, t_steps=T):
    """Host-side: compose weights, build wpack/bpack, per-core input shards."""
    f = np.float32
    asf = lambda x: np.array(x, f)
    (u, y, h0, pu_w1, pu_b1, pu_w2, pu_b2, dy_w1, dy_b1, dy_w2, dy_b2, xm_w,
     xm_b, xlv_w, xlv_b, px_w1, px_b1, px_w2, px_b2, me_w1, me_b1, me_w2,
     me_b2, gru_wih, gru_whh) = map(asf, (
         u, y, h0, pu_w1, pu_b1, pu_w2, pu_b2, dy_w1, dy_b1, dy_w2, dy_b2,
         xm_w, xm_b, xlv_w, xlv_b, px_w1, px_b1, px_w2, px_b2, me_w1, me_b1,
         me_w2, me_b2, gru_wih, gru_whh))
    wih0p, wih1p = gru_wih[0], gru_wih[1]
    whh0p, whh1p = gru_whh[0], gru_whh[1]
    # negated z-gate copies (sigmoid gives zc = 1-z)
    def zneg(w):
        w = w.copy()
        w[H:2 * H] *= -1.0
        return w
    wih0, wih1 = zneg(wih0p), zneg(wih1p)
    whh0, whh1 = zneg(whh0p), zneg(whh1p)

    Wg0 = wih0 @ pu_w2           # [3H, H], z rows negated
    bg0 = wih0 @ pu_b2
    Wg0p_z = wih0p[H:2 * H] @ pu_w2   # positive z
    bg0p_z = wih0p[H:2 * H] @ pu_b2
    Wda = dy_w1[:, :H] @ pu_w2
    bda = dy_w1[:, :H] @ pu_b2 + dy_b1
    dyw1b = dy_w1[:, H:]
    Wxc = np.vstack([xm_w, xlv_w])
    WxcP = Wxc @ dy_w2
    bxcP = Wxc @ dy_b2 + np.concatenate([xm_b, xlv_b])
    Wpx = px_w1 @ WxcP
    bpx = px_w1 @ bxcP + px_b1
    WmeP = me_w1 @ px_w2
    bmeP = me_w1 @ px_b2 + me_b1

    wpack = np.zeros((128, 23 * 128), f)

    def put(i, w):  # w: [out, in] -> lhsT [in, out]
        wt = np.ascontiguousarray(w.T)
        wpack[:wt.shape[0], i * 128:i * 128 + wt.shape[1]] = wt

    put(0, pu_w1)
    for g in range(3):
        put(1 + g, Wg0[g * H:(g + 1) * H])
        put(4 + g, whh0[g * H:(g + 1) * H])
        put(7 + g, wih1[g * H:(g + 1) * H])
        put(10 + g, whh1[g * H:(g + 1) * H])
    put(13, Wda)
    put(14, dyw1b)
    put(15, Wpx)
    put(16, WmeP)
    put(17, me_w2)
    put(18, np.eye(128, dtype=f))
    put(19, Wg0p_z)
    put(20, whh0p[H:2 * H])
    put(21, wih1p[H:2 * H])
    put(22, whh1p[H:2 * H])

    bpack = np.zeros((128, 10), f)
    for j, b in enumerate([pu_b1, bg0[:H], bg0[H:2 * H], bg0[2 * H:],
                           bda, bpx, bmeP, me_b2, bg0p_z]):
        bpack[:len(b), j] = b

    in_maps = []
    for c in range(NCORE):
        rows = slice(c * BL, (c + 1) * BL)
        in_maps.append({
            "u": _bf(u[rows, :, :t_steps].transpose(1, 2, 0)),  # [U, T, BL]
            "y": _bf(y[rows, :, :t_steps].transpose(1, 2, 0)),
            "h00": np.ascontiguousarray(h0[0, rows].T),
            "h01": np.ascontiguousarray(h0[1, rows].T),
            "wpack": _bf(wpack),
            "bpack": bpack,
        })
    return in_maps


_NC_CACHE = {}


def run_on_cores(in_maps, nch=NCH, trace=False, tmpdir=None):
    key = nch
    if key not in _NC_CACHE:
        _NC_CACHE[key] = build_nc(nch)
    nc = _NC_CACHE[key]
    res = run_bass_kernel_spmd(nc, in_maps, core_ids=list(range(NCORE)),
                               trace=trace, tmpdir=tmpdir)
    total = np.float32(0.0)
    for r in res.results:
        total += np.float32(r["out"][0, 0])
    return np.float32(total), res


def kernel(**inputs):
    in_maps = prep_inputs(**inputs)
    total, _ = run_on_cores(in_maps)
    return total


# revision 18
# speedup vs baseline: 1.4075x; 1.0989x over previous
"""Trainium2 Bass kernel for nn_AE_RNN: 2-layer GRU AE-RNN, scalar SSE loss.

Data-parallel over batch (256 -> 32 per core, 8 cores). Host sums partials.

v3: merged-layer lockstep recurrence. Iteration i computes layer0 step i
("A", cols 0:32) and layer1 step i-1 ("B", cols 32:64) in shared [128,64]
elementwise ops. Gates carry both a negated-z (zc = 1-z) and positive-z
(z') sigmoid so the blend h = zc*n + z'*h_prev needs no 1-x op. Gate
input biases preload into PSUM via identity matmuls; per-step chain is
  3 MMs -> sigmoid(r) -> tn -> tnb -> tanh -> aa -> h'
with the n-PSUM copied to SBUF off-chain so tn/tnb run in DVE 2x mode.
Bulk gi passes use DVE tensor_scalar (bias AP); tail ReLUs use DVE
scalar_tensor_tensor; bulk/tail emission is interleaved into the step
loop so engine FIFOs never stall at chunk boundaries.
"""

import numpy as np
import ml_dtypes

import concourse.bass as bass
import concourse.bacc as bacc
import concourse.mybir as mybir
import concourse.tile as tile
from concourse.bass_utils import run_bass_kernel_spmd

BF16 = mybir.dt.bfloat16
F32 = mybir.dt.float32
AF = mybir.ActivationFunctionType
OP = mybir.AluOpType

B, T = 256, 2048
U = Y = 32
H = 128
Z = 64
NCORE = 8
BL = B // NCORE  # 32 batch rows per core

C = 128  # steps per chunk
NCH = T // C


def _bf(x):
    return np.asarray(x, dtype=ml_dtypes.bfloat16)


def build_nc(nch=NCH):
    nc = bacc.Bacc("TRN2", target_bir_lowering=False, debug=False)
    Tl = nch * C
    NT = (C * BL) // 512  # 512-wide tiles per chunk = 8

    u_d = nc.dram_tensor("u", [U, Tl, BL], BF16, kind="ExternalInput")
    y_d = nc.dram_tensor("y", [Y, Tl, BL], BF16, kind="ExternalInput")
    h00_d = nc.dram_tensor("h00", [H, BL], F32, kind="ExternalInput")
    h01_d = nc.dram_tensor("h01", [H, BL], F32, kind="ExternalInput")
    wp_d = nc.dram_tensor("wpack", [128, 23 * 128], BF16, kind="ExternalInput")
    bp_d = nc.dram_tensor("bpack", [128, 10], F32, kind="ExternalInput")
    out_d = nc.dram_tensor("out", [1, 1], F32, kind="ExternalOutput")

    with tile.TileContext(nc) as tc:
        with (
            tc.tile_pool(name="wt", bufs=1) as wtp,
            tc.tile_pool(name="ut", bufs=2) as utp,
            tc.tile_pool(name="yt", bufs=3) as ytp,
            tc.tile_pool(name="big", bufs=2) as bigp,
            tc.tile_pool(name="r1", bufs=3) as r1p,
            tc.tile_pool(name="hs", bufs=3) as hsp,
            tc.tile_pool(name="sm", bufs=4) as smp,
            tc.tile_pool(name="tl", bufs=2) as tlp,
            tc.tile_pool(name="prr", bufs=2, space="PSUM") as prrp,
            tc.tile_pool(name="prz", bufs=2, space="PSUM") as przp,
            tc.tile_pool(name="pn", bufs=2, space="PSUM") as pnp,
            tc.tile_pool(name="psb", bufs=2, space="PSUM") as psb,
        ):
            wp = wtp.tile([128, 23 * 128], BF16)
            bp = wtp.tile([128, 10], F32)
            nc.sync.dma_start(wp[:], wp_d[:])
            nc.sync.dma_start(bp[:], bp_d[:])

            def W(i, n=128):
                return wp[:, i * 128:i * 128 + n]

            def Bc(j, p=128):
                return bp[:p, j:j + 1]

            I128 = W(18)

            ones32 = wtp.tile([32, 1], F32)
            nc.vector.memset(ones32[:], 1.0)
            zeros512 = wtp.tile([128, 512], BF16)
            nc.vector.memset(zeros512[:], 0.0)

            h0f = wtp.tile([H, BL], F32)
            h1f = wtp.tile([H, BL], F32)
            nc.sync.dma_start(h0f[:], h00_d[:])
            nc.sync.dma_start(h1f[:], h01_d[:])

            loss_acc = wtp.tile([32, NT * nch], F32)
            nc.vector.memset(loss_acc[:], 0.0)

            # ---------- bulk building blocks (emitted interleaved) ----------
            def relu1_tile(relu1, Ut, j):
                ps = psb.tile([128, 512], F32, tag="pb")
                nc.tensor.matmul(ps[:], W(0)[0:32, :],
                                 Ut[:, j * 512:(j + 1) * 512])
                nc.scalar.activation(relu1[:, j * 512:(j + 1) * 512], ps[:],
                                     AF.Relu, bias=Bc(0))

            def gi_tile(gb, relu1, wi, bi, j):
                ps = psb.tile([128, 512], F32, tag="pb")
                nc.tensor.matmul(ps[:], W(wi),
                                 relu1[:, j * 512:(j + 1) * 512])
                nc.vector.tensor_scalar(gb[:, j * 512:(j + 1) * 512], ps[:],
                                        Bc(bi), None, op0=OP.add)

            def tail_a(prev, j):
                relu1, HS_t, Yt = prev
                hsv = HS_t[:].rearrange("p (t x) -> p t x", x=64)
                sl = slice(j * 512, (j + 1) * 512)
                h1rhs = hsv[:, j * 16 + 1:j * 16 + 17, 32:64]
                ps = psb.tile([128, 512], F32, tag="pb")
                nc.tensor.matmul(ps[:], W(13), relu1[:, sl],
                                 start=True, stop=False)
                nc.tensor.matmul(ps[:], W(14), h1rhs, start=False, stop=True)
                dphi = tlp.tile([128, 512], BF16, tag="dphi")
                nc.vector.scalar_tensor_tensor(dphi[:], ps[:], Bc(4),
                                               zeros512[:], op0=OP.add,
                                               op1=OP.max)
                return dphi

            def tail_b(dphi):
                ps2 = psb.tile([128, 512], F32, tag="pb")
                nc.tensor.matmul(ps2[:], W(15), dphi[:])
                px = tlp.tile([128, 512], BF16, tag="px")
                nc.vector.scalar_tensor_tensor(px[:], ps2[:], Bc(5),
                                               zeros512[:], op0=OP.add,
                                               op1=OP.max)
                return px

            def tail_c(px):
                ps3 = psb.tile([128, 512], F32, tag="pb")
                nc.tensor.matmul(ps3[:], W(16), px[:])
                me = tlp.tile([128, 512], BF16, tag="me")
                nc.vector.scalar_tensor_tensor(me[:], ps3[:], Bc(6),
                                               zeros512[:], op0=OP.add,
                                               op1=OP.max)
                return me

            def tail_d(me, prev, k, j):
                _, _, Yt = prev
                sl = slice(j * 512, (j + 1) * 512)
                ps4t = psb.tile([128, 512], F32, tag="pb")
                ps4 = ps4t[0:32, :]
                nc.tensor.matmul(ps4, W(17, 32), me[:])
                d = tlp.tile([32, 512], F32, tag="dd")
                nc.vector.scalar_tensor_tensor(d[:], ps4, Bc(7, 32),
                                               Yt[:, sl], op0=OP.add,
                                               op1=OP.subtract)
                d2 = tlp.tile([32, 512], F32, tag="d2")
                nc.scalar.activation(
                    d2[:], d[:], AF.Square,
                    accum_out=loss_acc[:, k * NT + j:k * NT + j + 1])

            def tail_tile(prev, k, j):
                tail_d(tail_c(tail_b(tail_a(prev, j))), prev, k, j)

            # ---------- prologue: chunk 0 inputs + bulk ----------
            Ut = utp.tile([U, C * BL], BF16, tag="ut")
            Yt = ytp.tile([Y, C * BL], BF16, tag="yt")
            nc.sync.dma_start(Ut[:].rearrange("p (t b) -> p t b", b=BL),
                              u_d[:, 0:C, :])
            nc.sync.dma_start(Yt[:].rearrange("p (t b) -> p t b", b=BL),
                              y_d[:, 0:C, :])
            relu1 = r1p.tile([H, C * BL], BF16, tag="relu1")
            G0R = bigp.tile([H, C * BL], BF16, tag="g0r")
            G0Z = bigp.tile([H, C * BL], BF16, tag="g0z")
            G0P = bigp.tile([H, C * BL], BF16, tag="g0p")
            G0N = bigp.tile([H, C * BL], BF16, tag="g0n")
            for j in range(NT):
                relu1_tile(relu1, Ut, j)
            for j in range(NT):
                gi_tile(G0R, relu1, 1, 1, j)
                gi_tile(G0Z, relu1, 2, 2, j)
                gi_tile(G0P, relu1, 19, 8, j)
                gi_tile(G0N, relu1, 3, 3, j)

            HS_prev_tile = None
            prev = None  # (relu1, HS, Yt) of previous chunk for tail
            aap = bbp = None

            for k in range(nch):
                HS = hsp.tile([H, (C + 1) * 64], BF16, tag="hs")
                if k == 0:
                    nc.vector.tensor_copy(HS[:, 0:32], h0f[:])
                    nc.vector.tensor_copy(HS[:, 96:128], h1f[:])

                last = k + 1 >= nch
                if not last:
                    Ut_n = utp.tile([U, C * BL], BF16, tag="ut")
                    Yt_n = ytp.tile([Y, C * BL], BF16, tag="yt")
                    nc.sync.dma_start(
                        Ut_n[:].rearrange("p (t b) -> p t b", b=BL),
                        u_d[:, (k + 1) * C:(k + 2) * C, :])
                    nc.sync.dma_start(
                        Yt_n[:].rearrange("p (t b) -> p t b", b=BL),
                        y_d[:, (k + 1) * C:(k + 2) * C, :])
                    relu1_n = r1p.tile([H, C * BL], BF16, tag="relu1")
                    G0R_n = bigp.tile([H, C * BL], BF16, tag="g0r")
                    G0Z_n = bigp.tile([H, C * BL], BF16, tag="g0z")
                    G0P_n = bigp.tile([H, C * BL], BF16, tag="g0p")
                    G0N_n = bigp.tile([H, C * BL], BF16, tag="g0n")

                for s in range(C):
                    i = k * C + s  # global iteration
                    first = (i == 0)
                    if s == 0:
                        hprev = (HS[:, 0:64] if k == 0 else
                                 HS_prev_tile[:, C * 64:(C + 1) * 64])
                    else:
                        hprev = HS[:, s * 64:(s + 1) * 64]
                    h0p = hprev[:, 0:32]
                    h1p = hprev[:, 32:64]
                    s32 = slice(s * 32, (s + 1) * 32)

                    # --- PSUM rr: [rA rB]; rz: [zcA zcB zpA zpB] ---
                    rr = prrp.tile([128, 64], F32, tag="rr")
                    rz = przp.tile([128, 128], F32, tag="rz")
                    nps = pnp.tile([128, 128], F32, tag="n")
                    # preloads (no h dependency; run early)
                    nc.tensor.matmul(rr[:, 0:32], I128, G0R[:, s32],
                                     start=True, stop=False)
                    nc.tensor.matmul(rz[:, 0:32], I128, G0Z[:, s32],
                                     start=True, stop=False)
                    nc.tensor.matmul(rz[:, 64:96], I128, G0P[:, s32],
                                     start=True, stop=False)
                    nc.tensor.matmul(nps[:, 64:96], I128, G0N[:, s32])
                    # h-dependent: r first (chain), then n, then z.
                    # For i>=2 the r-gate consumes aa/bb of the previous step
                    # directly (W@h' = W@aa + W@bb) so the h'-add is off-chain.
                    if i >= 2:
                        nc.tensor.matmul(rr[:, 0:32], W(4), aap[:, 0:32],
                                         start=False, stop=False)
                        nc.tensor.matmul(rr[:, 0:32], W(4), bbp[:, 0:32],
                                         start=False, stop=True)
                        nc.tensor.matmul(rr[:, 32:64], W(7), aap[:, 0:32],
                                         start=True, stop=False)
                        nc.tensor.matmul(rr[:, 32:64], W(7), bbp[:, 0:32],
                                         start=False, stop=False)
                        nc.tensor.matmul(rr[:, 32:64], W(10), aap[:, 32:64],
                                         start=False, stop=False)
                        nc.tensor.matmul(rr[:, 32:64], W(10), bbp[:, 32:64],
                                         start=False, stop=True)
                    else:
                        nc.tensor.matmul(rr[:, 0:32], W(4), h0p,
                                         start=False, stop=True)
                        if not first:
                            nc.tensor.matmul(rr[:, 32:64], W(7), h0p,
                                             start=True, stop=False)
                            nc.tensor.matmul(rr[:, 32:64], W(10), h1p,
                                             start=False, stop=True)
                    nc.tensor.matmul(nps[:, 0:32], W(6), h0p)
                    if not first:
                        nc.tensor.matmul(nps[:, 32:64], W(12), h1p)
                        nc.tensor.matmul(nps[:, 96:128], W(9), h0p)
                    nc.tensor.matmul(rz[:, 0:32], W(5), h0p,
                                     start=False, stop=True)
                    nc.tensor.matmul(rz[:, 64:96], W(20), h0p,
                                     start=False, stop=True)
                    if not first:
                        nc.tensor.matmul(rz[:, 32:64], W(8), h0p,
                                         start=True, stop=False)
                        nc.tensor.matmul(rz[:, 32:64], W(11), h1p,
                                         start=False, stop=True)
                        nc.tensor.matmul(rz[:, 96:128], W(21), h0p,
                                         start=True, stop=False)
                        nc.tensor.matmul(rz[:, 96:128], W(22), h1p,
                                         start=False, stop=True)

                    wd = 32 if first else 64
                    # --- sigmoids ---
                    rs = smp.tile([128, 64], BF16, tag="rs")
                    nc.scalar.activation(rs[:, 0:wd], rr[:, 0:wd], AF.Sigmoid)
                    zs = smp.tile([128, 128], BF16, tag="zs")
                    if first:
                        nc.scalar.activation(zs[:, 0:32], rz[:, 0:32],
                                             AF.Sigmoid)
                        nc.scalar.activation(zs[:, 64:96], rz[:, 64:96],
                                             AF.Sigmoid)
                    else:
                        nc.scalar.activation(zs[:], rz[:], AF.Sigmoid)
                    # --- n path ---
                    tn = smp.tile([128, 64], BF16, tag="tn")
                    nc.vector.tensor_tensor(tn[:, 0:wd], rs[:, 0:wd],
                                            nps[:, 0:wd], op=OP.mult)
                    tnb = smp.tile([128, 64], BF16, tag="tnb")
                    nc.vector.tensor_tensor(tnb[:, 0:wd], tn[:, 0:wd],
                                            nps[:, 64:64 + wd], op=OP.add)
                    ns = smp.tile([128, 64], BF16, tag="ns")
                    nc.scalar.activation(ns[:, 0:wd], tnb[:, 0:wd], AF.Tanh)
                    # --- blend: h = zc*n + z'*h_prev ---
                    bb = smp.tile([128, 64], BF16, tag="bb")
                    nc.gpsimd.tensor_tensor(bb[:, 0:wd], zs[:, 64:64 + wd],
                                            hprev[:, 0:wd], op=OP.mult)
                    aa = smp.tile([128, 64], BF16, tag="aa")
                    nc.vector.tensor_tensor(aa[:, 0:wd], zs[:, 0:wd],
                                            ns[:, 0:wd], op=OP.mult)
                    dst = HS[:, (s + 1) * 64:(s + 1) * 64 + wd]
                    nc.vector.tensor_tensor(dst, aa[:, 0:wd], bb[:, 0:wd],
                                            op=OP.add)
                    aap, bbp = aa, bb

                    # --- interleaved bulk (chunk k+1) and tail (chunk k-1) ---
                    j, ph = divmod(s - 4, 16)
                    if 0 <= j < NT:
                        if ph == 0 and not last:
                            relu1_tile(relu1_n, Ut_n, j)
                        elif ph == 2 and not last:
                            gi_tile(G0R_n, relu1_n, 1, 1, j)
                        elif ph == 4 and not last:
                            gi_tile(G0Z_n, relu1_n, 2, 2, j)
                        elif ph == 6 and not last:
                            gi_tile(G0P_n, relu1_n, 19, 8, j)
                        elif ph == 8 and not last:
                            gi_tile(G0N_n, relu1_n, 3, 3, j)
                        elif ph == 9 and prev is not None:
                            t_dphi = tail_a(prev, j)
                        elif ph == 11 and prev is not None:
                            t_px = tail_b(t_dphi)
                        elif ph == 13 and prev is not None:
                            t_me = tail_c(t_px)  # never fires for j=NT-1
                        elif ph == 15 and prev is not None:
                            tail_d(t_me, prev, k - 1, j)

                # last tile's tail_c/tail_d fall past s=127; emit here
                if prev is not None:
                    t_me = tail_c(t_px)
                    tail_d(t_me, prev, k - 1, NT - 1)

                prev = (relu1, HS, Yt)
                HS_prev_tile = HS
                if not last:
                    relu1 = relu1_n
                    G0R, G0Z, G0P, G0N = G0R_n, G0Z_n, G0P_n, G0N_n
                    Ut, Yt = Ut_n, Yt_n

            for j in range(NT):
                tail_tile(prev, nch - 1, j)

            # ---- final loss reduction ----
            lsum = smp.tile([32, 1], F32, tag="lsum")
            nc.vector.tensor_reduce(lsum[:], loss_acc[:],
                                    axis=mybir.AxisListType.X, op=OP.add)
            pslt = psb.tile([128, 512], F32, tag="pb")
            psl = pslt[0:1, 0:1]
            nc.tensor.matmul(psl, ones32[:], lsum[:])
            lout = smp.tile([1, 1], F32, tag="lout")
            nc.vector.tensor_copy(lout[:], psl)
            nc.sync.dma_start(out_d[:], lout[:])

    nc.compile()
    return nc


def prep_inputs(u, y, h0, pu_w1, pu_b1, pu_w2, pu_b2, dy_w1, dy_b1, dy_w2,
                dy_b2, xm_w, xm_b, xlv_w, xlv_b, px_w1, px_b1, px_w2, px_b2,
                me_w1, me_b1, me_w2, me_b2, gru_wih, gru_whh, t_steps=T):
    """Host-side: compose weights, build wpack/bpack, per-core input shards."""
    f = np.float32
    asf = lambda x: np.array(x, f)
    (u, y, h0, pu_w1, pu_b1, pu_w2, pu_b2, dy_w1, dy_b1, dy_w2, dy_b2, xm_w,
     xm_b, xlv_w, xlv_b, px_w1, px_b1, px_w2, px_b2, me_w1, me_b1, me_w2,
     me_b2, gru_wih, gru_whh) = map(asf, (
         u, y, h0, pu_w1, pu_b1, pu_w2, pu_b2, dy_w1, dy_b1, dy_w2, dy_b2,
         xm_w, xm_b, xlv_w, xlv_b, px_w1, px_b1, px_w2, px_b2, me_w1, me_b1,
         me_w2, me_b2, gru_wih, gru_whh))
    wih0p, wih1p = gru_wih[0], gru_wih[1]
    whh0p, whh1p = gru_whh[0], gru_whh[1]
    # negated z-gate copies (sigmoid gives zc = 1-z)
    def zneg(w):
        w = w.copy()
        w[H:2 * H] *= -1.0
        return w
    wih0, wih1 = zneg(wih0p), zneg(wih1p)
    whh0, whh1 = zneg(whh0p), zneg(whh1p)

    Wg0 = wih0 @ pu_w2           # [3H, H], z rows negated
    bg0 = wih0 @ pu_b2
    Wg0p_z = wih0p[H:2 * H] @ pu_w2   # positive z
    bg0p_z = wih0p[H:2 * H] @ pu_b2
    Wda = dy_w1[:, :H] @ pu_w2
    bda = dy_w1[:, :H] @ pu_b2 + dy_b1
    dyw1b = dy_w1[:, H:]
    Wxc = np.vstack([xm_w, xlv_w])
    WxcP = Wxc @ dy_w2
    bxcP = Wxc @ dy_b2 + np.concatenate([xm_b, xlv_b])
    Wpx = px_w1 @ WxcP
    bpx = px_w1 @ bxcP + px_b1
    WmeP = me_w1 @ px_w2
    bmeP = me_w1 @ px_b2 + me_b1

    wpack = np.zeros((128, 23 * 128), f)

    def put(i, w):  # w: [out, in] -> lhsT [in, out]
        wt = np.ascontiguousarray(w.T)
        wpack[:wt.shape[0], i * 128:i * 128 + wt.shape[1]] = wt

    put(0, pu_w1)
    for g in range(3):
        put(1 + g, Wg0[g * H:(g + 1) * H])
        put(4 + g, whh0[g * H:(g + 1) * H])
        put(7 + g, wih1[g * H:(g + 1) * H])
        put(10 + g, whh1[g * H:(g + 1) * H])
    put(13, Wda)
    put(14, dyw1b)
    put(15, Wpx)
    put(16, WmeP)
    put(17, me_w2)
    put(18, np.eye(128, dtype=f))
    put(19, Wg0p_z)
    put(20, whh0p[H:2 * H])
    put(21, wih1p[H:2 * H])
    put(22, whh1p[H:2 * H])

    bpack = np.zeros((128, 10), f)
    for j, b in enumerate([pu_b1, bg0[:H], bg0[H:2 * H], bg0[2 * H:],
                           bda, bpx, bmeP, me_b2, bg0p_z]):
        bpack[:len(b), j] = b

    in_maps = []
    for c in range(NCORE):
        rows = slice(c * BL, (c + 1) * BL)
        in_maps.append({
            "u": _bf(u[rows, :, :t_steps].transpose(1, 2, 0)),  # [U, T, BL]
            "y": _bf(y[rows, :, :t_steps].transpose(1, 2, 0)),
            "h00": np.ascontiguousarray(h0[0, rows].T),
            "h01": np.ascontiguousarray(h0[1, rows].T),
            "wpack": _bf(wpack),
            "bpack": bpack,
        })
    return in_maps


_NC_CACHE = {}


def run_on_cores(in_maps, nch=NCH, trace=False, tmpdir=None):
    key = nch
    if key not in _NC_CACHE:
        _NC_CACHE[key] = build_nc(nch)
    nc = _NC_CACHE[key]
    res = run_bass_kernel_spmd(nc, in_maps, core_ids=list(range(NCORE)),
                               trace=trace, tmpdir=tmpdir)
    total = np.float32(0.0)
    for r in res.results:
        total += np.float32(r["out"][0, 0])
    return np.float32(total), res


def kernel(**inputs):
    in_maps = prep_inputs(**inputs)
    total, _ = run_on_cores(in_maps)
    return total
